# revision 13
# baseline (speedup 1.0000x reference)
"""Trainium2 Bass kernel for a dense attention layer (8 NeuronCores).

reference math (per batch b):
  q = queries @ Wq + bq ; k = keys @ Wk + bk ; v = values @ Wv + bv
  scores = einsum("blhe,bshe->bhls", q, k) * tau[b] + delta[b][None,None,:]
  A = softmax(scores / sqrt(E), axis=-1)
  out = einsum("bhls,bshd->blhd", A, v) @ Wo + bo
  returns (out, A)

Sharding: 8 cores = 2 batches x 4 head-groups (4 heads each).
Core c handles batch c//4, heads 4*(c%4) .. 4*(c%4)+3:
  - QKV weight columns [256*g : 256*(g+1)] (column / tensor parallel)
  - Wo rows [256*g : 256*(g+1)] -> partial out, summed on host
  - A^T (unnormalized exp of logits) written per head; host transposes
    and normalizes with the device-computed row sums.
No collectives needed; host does the (cheap) final reduction.
"""

import sys

sys.path.insert(0, "/opt/trn_rl_repo")

import numpy as np

import concourse.bass as bass
import concourse.tile as tile
from concourse import bacc, mybir
from concourse.masks import make_identity

FP32 = mybir.dt.float32
BF16 = mybir.dt.bfloat16
AF = mybir.ActivationFunctionType
ALU = mybir.AluOpType

P = 128  # partitions


def _bcast_ap(ap, parts, free):
    """DRAM AP broadcast: [[0, parts], [1, free]] starting at ap's offset."""
    return bass.AP(tensor=ap.tensor, offset=ap.offset, ap=[[0, parts], [1, free]])


def build_attention_nc(L=2048, S=2048, D=1024, H_LOC=4, E=64, n_devices=8):
    """Build the per-core Bass graph (SPMD: same graph on all cores)."""
    EH = H_LOC * E  # local e columns (256)
    N_ST = S // P  # s tiles (16)
    N_LT = L // P  # l tiles (16)
    N_DC = D // P  # contraction chunks (8)
    LCH = min(512, L)  # matmul free-dim chunk
    N_LC = L // LCH
    SCH = min(512, S)
    ET = EH // P  # e tiles of 128 (2)
    assert EH % P == 0 and E == 64 and L % LCH == 0

    nc = bacc.Bacc(
        "TRN2", target_bir_lowering=False, debug=False, num_devices=n_devices
    )

    xq = nc.dram_tensor("xq", [L, D], FP32, kind="ExternalInput").ap()
    xk = nc.dram_tensor("xk", [S, D], FP32, kind="ExternalInput").ap()
    xv = nc.dram_tensor("xv", [S, D], FP32, kind="ExternalInput").ap()
    wq = nc.dram_tensor("wq", [D, EH], FP32, kind="ExternalInput").ap()
    wk = nc.dram_tensor("wk", [D, EH], FP32, kind="ExternalInput").ap()
    wv = nc.dram_tensor("wv", [D, EH], FP32, kind="ExternalInput").ap()
    wo = nc.dram_tensor("wo", [EH, D], FP32, kind="ExternalInput").ap()
    bq = nc.dram_tensor("bq", [EH], FP32, kind="ExternalInput").ap()
    bk = nc.dram_tensor("bk", [EH], FP32, kind="ExternalInput").ap()
    bv = nc.dram_tensor("bv", [EH], FP32, kind="ExternalInput").ap()
    tau = nc.dram_tensor("tau", [1], FP32, kind="ExternalInput").ap()
    delta = nc.dram_tensor("delta", [S], FP32, kind="ExternalInput").ap()

    a_t = nc.dram_tensor("a_t", [H_LOC, S, L], FP32, kind="ExternalOutput").ap()
    out_p = nc.dram_tensor("out_p", [L, D], FP32, kind="ExternalOutput").ap()
    rsum = nc.dram_tensor("rsum", [H_LOC, L], FP32, kind="ExternalOutput").ap()

    inv_sqrt_e = 1.0 / float(np.sqrt(E))

    with tile.TileContext(nc) as tc:
        with (
            tc.tile_pool(name="singles", bufs=1) as singles,
            tc.tile_pool(name="xload", bufs=2) as xload,
            tc.tile_pool(name="xt", bufs=2) as xtp,
            tc.tile_pool(name="pbf", bufs=4) as pbfp,
            tc.tile_pool(name="osb", bufs=3) as osbp,
            tc.tile_pool(name="small", bufs=4) as smallp,
            tc.tile_pool(name="dram", bufs=4, space="DRAM") as dramp,
            tc.tile_pool(name="ps_big", bufs=2, space="PSUM") as ps_big,
            tc.tile_pool(name="ps_pv", bufs=4, space="PSUM") as ps_pv,
        ):
            # ---- constants / small loads ----
            ident = singles.tile([P, P], BF16, tag="ident")
            make_identity(nc, ident)

            tau_sb = singles.tile([P, 1], FP32, tag="tau")
            nc.gpsimd.dma_start(out=tau_sb, in_=_bcast_ap(tau, P, 1))
            nc.scalar.mul(tau_sb, tau_sb, inv_sqrt_e)  # (1/sqrt(E)) * tau

            delta_sb = singles.tile([P, N_ST], FP32, tag="delta")
            nc.gpsimd.dma_start(
                out=delta_sb, in_=delta.rearrange("(t p) -> p t", p=P)
            )
            nc.scalar.mul(delta_sb, delta_sb, inv_sqrt_e)

            bq_sb = singles.tile([P, ET], FP32, tag="bq")
            nc.gpsimd.dma_start(out=bq_sb, in_=bq.rearrange("(c p) -> p c", p=P))
            bk_sb = singles.tile([P, ET], FP32, tag="bk")
            nc.gpsimd.dma_start(out=bk_sb, in_=bk.rearrange("(c p) -> p c", p=P))
            bvb = singles.tile([P, EH], FP32, tag="bvb")
            nc.gpsimd.dma_start(out=bvb, in_=_bcast_ap(bv, P, EH))

            wq_sb = singles.tile([P, N_DC, EH], BF16, tag="wq")
            nc.gpsimd.dma_start(
                out=wq_sb, in_=wq.rearrange("(c p) e -> p c e", p=P)
            )
            wk_sb = singles.tile([P, N_DC, EH], BF16, tag="wk")
            nc.gpsimd.dma_start(
                out=wk_sb, in_=wk.rearrange("(c p) e -> p c e", p=P)
            )
            wv_sb = singles.tile([P, N_DC, EH], BF16, tag="wv")
            nc.gpsimd.dma_start(
                out=wv_sb, in_=wv.rearrange("(c p) e -> p c e", p=P)
            )
            wo_sb = singles.tile([P, ET, D], BF16, tag="wo")
            nc.gpsimd.dma_start(
                out=wo_sb, in_=wo.rearrange("(c p) n -> p c n", p=P)
            )

            # persistent activations
            qT = singles.tile([P, ET, L], BF16, tag="qT")  # [e, l] per e-tile
            kT = singles.tile([P, ET, S], BF16, tag="kT")
            vsb = singles.tile([P, N_ST, H_LOC, E + 1], BF16, tag="vsb")
            nc.vector.memset(vsb, 1.0)  # col E stays 1.0 -> row-sum trick
            attnT = singles.tile([P, ET, L], BF16, tag="attnT")

            # ---- stage 1: transpose X chunks and project ----
            def load_and_transpose(x_ext, lc, nrows):
                """Load rows [lc*LCH, +LCH) of x_ext (cast f32->bf16) and
                build x^T chunk [P(d), N_DC, LCH(l)]."""
                x_t = xload.tile([P, LCH // P, D], BF16, tag="x")
                nc.gpsimd.dma_start(
                    out=x_t,
                    in_=x_ext[lc * LCH : (lc + 1) * LCH, :].rearrange(
                        "(i p) d -> p i d", p=P
                    ),
                )
                xt_c = xtp.tile([P, N_DC, LCH], BF16, tag="xt")
                for dc in range(N_DC):
                    ps_t = ps_big.tile([P, LCH], BF16, tag="ps")
                    for i in range(LCH // P):
                        nc.tensor.transpose(
                            ps_t[:, i * P : (i + 1) * P],
                            x_t[:, i, dc * P : (dc + 1) * P],
                            ident,
                        )
                    nc.any.tensor_copy(out=xt_c[:, dc, :], in_=ps_t)
                return xt_c

            # q and k -> transposed layout [e, l], bias added, q scaled
            for x_ext, w_sb, dest, bias_sb, scaled in (
                (xq, wq_sb, qT, bq_sb, True),
                (xk, wk_sb, kT, bk_sb, False),
            ):
                for lc in range(N_LC):
                    xt_c = load_and_transpose(x_ext, lc, LCH)
                    for et in range(ET):
                        ps_q = ps_big.tile([P, LCH], FP32, tag="ps")
                        for dc in range(N_DC):
                            nc.tensor.matmul(
                                ps_q,
                                lhsT=w_sb[:, dc, et * P : (et + 1) * P],
                                rhs=xt_c[:, dc, :],
                                start=(dc == 0),
                                stop=(dc == N_DC - 1),
                            )
                        dst = dest[:, et, lc * LCH : (lc + 1) * LCH]
                        if scaled:
                            nc.vector.tensor_scalar(
                                out=dst,
                                in0=ps_q,
                                scalar1=bias_sb[:, et : et + 1],
                                scalar2=tau_sb,
                                op0=ALU.add,
                                op1=ALU.mult,
                            )
                        else:
                            nc.vector.tensor_scalar_add(
                                out=dst,
                                in0=ps_q,
                                scalar1=bias_sb[:, et : et + 1],
                            )

            # v -> natural layout [s, (h, e)] with ones column at e=E
            for lc in range(N_LC):
                xt_c = load_and_transpose(xv, lc, LCH)
                for sq in range(LCH // P):
                    st = lc * (LCH // P) + sq
                    ps_v = ps_big.tile([P, EH], FP32, tag="ps")
                    for dc in range(N_DC):
                        nc.tensor.matmul(
                            ps_v,
                            lhsT=xt_c[:, dc, sq * P : (sq + 1) * P],
                            rhs=wv_sb[:, dc, :],
                            start=(dc == 0),
                            stop=(dc == N_DC - 1),
                        )
                    nc.vector.tensor_add(
                        out=vsb[:, st, :, 0:E],
                        in0=ps_v.rearrange("p (h e) -> p h e", e=E),
                        in1=bvb.rearrange("p (h e) -> p h e", e=E),
                    )

            # ---- stage 2: per-head attention ----
            for h in range(H_LOC):
                et, hp = divmod(h, 2) if ET == 2 else (0, h)
                pv_ps = [
                    ps_pv.tile([E + 1, LCH], FP32, tag="pv", name=f"pv_{h}_{i}")
                    for i in range(N_LC)
                ]
                for st in range(N_ST):
                    p_bf = pbfp.tile([P, L], BF16, tag="pbf")
                    for l0 in range(0, L, 1024):
                        w = min(1024, L - l0)
                        ps_s = ps_big.tile([P, w], FP32, tag="ps")
                        for j0 in range(0, w, LCH):
                            nc.tensor.matmul(
                                ps_s[:, j0 : j0 + LCH],
                                lhsT=kT[
                                    hp * E : (hp + 1) * E,
                                    et,
                                    st * P : (st + 1) * P,
                                ],
                                rhs=qT[
                                    hp * E : (hp + 1) * E, et, l0 + j0 : l0 + j0 + LCH
                                ],
                                start=True,
                                stop=True,
                            )
                        # p = exp(q.k * tau/sqrt(E) + delta/sqrt(E)), bf16
                        nc.scalar.activation(
                            out=p_bf[:, l0 : l0 + w],
                            in_=ps_s,
                            func=AF.Exp,
                            bias=delta_sb[:, st : st + 1],
                            scale=1.0,
                        )
                    # unnormalized A^T out (cast bf16 -> f32 in DMA)
                    nc.gpsimd.dma_start(
                        out=a_t[h, st * P : (st + 1) * P, :], in_=p_bf
                    )
                    for lc4 in range(N_LC):
                        nc.tensor.matmul(
                            pv_ps[lc4],
                            lhsT=vsb[:, st, h, :],
                            rhs=p_bf[:, lc4 * LCH : (lc4 + 1) * LCH],
                            start=(st == 0),
                            stop=(st == N_ST - 1),
                        )
                # normalize -> attnT (bf16) and export row sums
                for lc4 in range(N_LC):
                    rs_t = smallp.tile([1, LCH], FP32, tag="rs")
                    nc.vector.tensor_copy(out=rs_t, in_=pv_ps[lc4][E : E + 1, :])
                    nc.sync.dma_start(
                        out=rsum[h, lc4 * LCH : (lc4 + 1) * LCH],
                        in_=rs_t,
                    )
                    rec = smallp.tile([1, LCH], FP32, tag="rec")
                    nc.vector.reciprocal(rec, pv_ps[lc4][E : E + 1, :])
                    rec_d = dramp.tile([1, LCH], FP32, tag="recd")
                    nc.sync.dma_start(out=rec_d, in_=rec)
                    rec_b = smallp.tile([E, LCH], FP32, tag="recb")
                    nc.gpsimd.dma_start(
                        out=rec_b, in_=_bcast_ap(rec_d, E, LCH)
                    )
                    nc.vector.tensor_mul(
                        out=attnT[
                            hp * E : (hp + 1) * E, et, lc4 * LCH : (lc4 + 1) * LCH
                        ],
                        in0=pv_ps[lc4][0:E, :],
                        in1=rec_b,
                    )

            # ---- stage 3: output projection (partial; host sums cores) ----
            OCH = min(LCH, D)
            for lt in range(N_LT):
                for n0 in range(0, D, OCH):
                    ps_o = ps_big.tile([P, OCH], FP32, tag="ps")
                    for ec in range(ET):
                        nc.tensor.matmul(
                            ps_o,
                            lhsT=attnT[:, ec, lt * P : (lt + 1) * P],
                            rhs=wo_sb[:, ec, n0 : n0 + OCH],
                            start=(ec == 0),
                            stop=(ec == ET - 1),
                        )
                    o_sb = osbp.tile([P, OCH], FP32, tag="osb")
                    nc.vector.tensor_copy(o_sb, ps_o)
                    nc.sync.dma_start(
                        out=out_p[lt * P : (lt + 1) * P, n0 : n0 + OCH], in_=o_sb
                    )

    nc.compile()
    return nc


_NC_CACHE = {}


def _get_nc():
    key = "full"
    if key not in _NC_CACHE:
        _NC_CACHE[key] = build_attention_nc()
    return _NC_CACHE[key]


def kernel(**inputs):
    out, A, _ = _run(inputs, trace=False)
    return out, A


def _run(inputs, trace=False):
    from concourse.bass_utils import run_bass_kernel_spmd

    q = np.asarray(inputs["queries"], dtype=np.float32)
    k = np.asarray(inputs["keys"], dtype=np.float32)
    v = np.asarray(inputs["values"], dtype=np.float32)
    tau = np.asarray(inputs["tau"], dtype=np.float32)
    delta = np.asarray(inputs["delta"], dtype=np.float32)
    Wq = np.asarray(inputs["Wq"], dtype=np.float32)
    bq = np.asarray(inputs["bq"], dtype=np.float32)
    Wk = np.asarray(inputs["Wk"], dtype=np.float32)
    bk = np.asarray(inputs["bk"], dtype=np.float32)
    Wv = np.asarray(inputs["Wv"], dtype=np.float32)
    bv = np.asarray(inputs["bv"], dtype=np.float32)
    Wo = np.asarray(inputs["Wo"], dtype=np.float32)
    bo = np.asarray(inputs["bo"], dtype=np.float32)

    B, L, Dm = q.shape
    S = k.shape[1]
    H = 16
    H_LOC = 4
    EH = Dm // H * H_LOC  # 256

    nc = _get_nc()
    in_maps = []
    for c in range(8):
        b, g = divmod(c, 4)
        cs = slice(g * EH, (g + 1) * EH)
        in_maps.append(
            {
                "xq": np.ascontiguousarray(q[b]),
                "xk": np.ascontiguousarray(k[b]),
                "xv": np.ascontiguousarray(v[b]),
                "wq": np.ascontiguousarray(Wq[:, cs]),
                "wk": np.ascontiguousarray(Wk[:, cs]),
                "wv": np.ascontiguousarray(Wv[:, cs]),
                "wo": np.ascontiguousarray(Wo[cs, :]),
                "bq": np.ascontiguousarray(bq[cs]),
                "bk": np.ascontiguousarray(bk[cs]),
                "bv": np.ascontiguousarray(bv[cs]),
                "tau": np.ascontiguousarray(tau[b]),
                "delta": np.ascontiguousarray(delta[b]),
            }
        )

    kr = run_bass_kernel_spmd(nc, in_maps, core_ids=list(range(8)), trace=trace)
    res = kr.results

    out = np.zeros((B, L, Dm), dtype=np.float32)
    A = np.empty((B, H, L, S), dtype=np.float32)
    for c in range(8):
        b, g = divmod(c, 4)
        out[b] += res[c]["out_p"]
        at = res[c]["a_t"]  # [H_LOC, S, L] unnormalized exp
        rs = res[c]["rsum"]  # [H_LOC, L]
        A[b, H_LOC * g : H_LOC * (g + 1)] = at.transpose(0, 2, 1) / rs[:, :, None]
    out += bo
    return out, A, kr
